# revision 1
# baseline (speedup 1.0000x reference)
"""CapsuleLayer (dynamic routing, 3 iterations) Trainium2 Bass kernel — v2.

Full inputs:  input_vectors [32, 2048, 16] f32, weight_matrix [1, 64, 32, 16] f32
Full output:  [32, 64, 32] f32

Sharding: data-parallel over batch; each of 8 NeuronCores processes 4 batches.
No collectives.

n-to-SBUF mapping: n = p*16 + j  (p = partition, j = strip 0..15) so the input
DMA moves 1KB-contiguous blocks per (p, b) instead of 64B gathers.

Algorithm (never materializes u = [B,N,O,D]):
  xs      = squash(x)          squash scale g = n/(eps+n2) (1e-8 negligible)
  iter 0: t0 = (1/64) sum_n xs          (one ones lhsT, 32 MMs, 1 LDW)
  iter k: logits = xs @ wv.T            (4 PE quads: batch b on array rows b*32,
                                         stationary xsT strips, rhs = trc)
          e = exp(logits) bf16;  Z = sum_o e;  xz = xs * (1/Z)
          t[o,(b,i)] = sum_n e*xz       (paired lhsT [e_b0|e_b1]: 1 LDW / 2 MMs;
                                         valid rows: b even 0:64, b odd 64:128)
  small stage in [128-part, (q=b//2, i)]: rows 0:64 hold b even, 64:128 b odd;
  wv = h*(M2 @ t) with host Gram M2 = W^T W;  trc_q = PE-transpose of wv.
  output  v = h * (W @ t) on both partition halves at the last iteration.
Iteration-2 logits rhs = wv0+wv1 (linearity) -> no cross-iteration PSUM state.

sqrt runs on DVE via pow(x,0.5) (CAPS_SQRT_MODE=lnexp falls back to
exp(0.5*ln x) on ACT); scalar engine stays on a single act table either way.
PE warmup matmuls run during the input-DMA window so HAM is at 2.4 GHz when
real matmuls start.
"""

import os

os.environ.setdefault("MYCRO_LOCAL_CACHE", "1")

import numpy as np
import ml_dtypes

import concourse.bass as bass
import concourse.tile as tile
from concourse import bacc, mybir
from concourse.bass_utils import run_bass_kernel_spmd

AF = mybir.ActivationFunctionType
ALU = mybir.AluOpType
F32 = mybir.dt.float32
BF16 = mybir.dt.float16  # "half" dtype: fp16 (same engine rates as bf16, more mantissa)

N_CORES = 8
B = 4          # batches per core
N = 2048       # input capsules
O = 64         # output capsules
DI = 16        # input capsule dim
D = 32         # output capsule dim
J = 16         # n-strips per batch (n = p*16 + j)
EPS = 0.5

SQRT_MODE = os.environ.get("CAPS_SQRT_MODE", "pow")   # "pow" | "lnexp"
MIXED_TT = int(os.environ.get("CAPS_MIXED_TT", "1"))  # allow bf16 x f32 tensor_tensor
WARMUP_MMS = int(os.environ.get("CAPS_WARMUP_MMS", "40"))
DEBUG_DUMP = os.environ.get("CAPS_DEBUG_DUMP", "")    # xs|xsT|t0|wv0|trc0|L1|e1|t1


def build_kernel(nc: bass.Bass, tc: tile.TileContext):
    from contextlib import ExitStack
    ctx = ExitStack()
    x = nc.dram_tensor("x", [B, N, DI], F32, kind="ExternalInput").ap()
    wrep = nc.dram_tensor("wrep", [128, D * DI], BF16, kind="ExternalInput").ap()
    m2rep = nc.dram_tensor("m2rep", [128, DI * DI], BF16, kind="ExternalInput").ap()
    ident = nc.dram_tensor("ident", [128, 128], BF16, kind="ExternalInput").ap()
    vout = nc.dram_tensor("vout", [B, O, D], F32, kind="ExternalOutput").ap()

    const = ctx.enter_context(tc.tile_pool(name="const", bufs=1))
    big = ctx.enter_context(tc.tile_pool(name="big", bufs=1))
    small = ctx.enter_context(tc.tile_pool(name="small", bufs=2))
    psumL = ctx.enter_context(tc.tile_pool(name="psumL", bufs=2, space="PSUM"))
    psumT = ctx.enter_context(tc.tile_pool(name="psumT", bufs=1, space="PSUM"))
    psumX = ctx.enter_context(tc.tile_pool(name="psumX", bufs=2, space="PSUM"))
    psumW = ctx.enter_context(tc.tile_pool(name="psumW", bufs=1, space="PSUM"))

    def squash_scale(out, n2, tag):
        # out = sqrt(n2)/(eps+n2).  The DVE ISA has no pow/rsqrt/divide, so
        # sqrt runs on ACT as exp(0.5*ln(x)) — ln/exp/square/copy share one act
        # table — and the division is a native DVE reciprocal + multiply.
        # The d/recip chain (DVE) runs in parallel with ln/exp (ACT).
        d = small.tile(list(n2.shape), F32, tag=f"{tag}_d")
        nc.vector.tensor_scalar_add(d[:], n2, EPS)
        rd = small.tile(list(n2.shape), F32, tag=f"{tag}_rd")
        nc.vector.reciprocal(rd[:], d[:])
        ln2 = small.tile(list(n2.shape), F32, tag=f"{tag}_ln2")
        nc.scalar.activation(ln2[:], n2, AF.Ln)
        nx = small.tile(list(n2.shape), F32, tag=f"{tag}_nx")
        nc.scalar.activation(nx[:], ln2[:], AF.Exp, 0.0, 0.5)
        nc.vector.tensor_mul(out, nx[:], rd[:])

    def dump_stop(src, note=""):
        # stage src ([P, C] any dtype, P<=128, C<=64) into f32 and write vout:
        # vout[b] = stage[(b%2)*64:(b%2)*64+64, (b//2)*32:(b//2)*32+32]
        stage = const.tile([128, 64], F32, tag="dumpstage")
        nc.gpsimd.memset(stage[:], 0.0)
        P, C = src.shape[0], src.shape[1]
        nc.vector.tensor_copy(stage[0:P, 0:C], src)
        for b in range(B):
            nc.sync.dma_start(
                vout[b],
                stage[(b % 2) * 64:(b % 2) * 64 + 64, (b // 2) * D:(b // 2 + 1) * D],
            )
        ctx.close()

    # ---- constants (ident first: warmup needs it) ----
    id_sb = const.tile([128, 128], BF16, tag="id_sb")
    nc.sync.dma_start(id_sb[:], ident)
    # input x: 1KB contiguous per (p, b)
    xr = big.tile([128, B * J * DI], F32, tag="xr")
    nc.sync.dma_start(
        xr[:].rearrange("p (b j i) -> p b j i", b=B, j=J),
        x.rearrange("b (p j) i -> p b j i", p=128),
    )
    w_sb = const.tile([128, D * DI], BF16, tag="w_sb")
    m2_sb = const.tile([128, DI * DI], BF16, tag="m2_sb")
    nc.sync.dma_start(w_sb[:], wrep)
    nc.sync.dma_start(m2_sb[:], m2rep)
    ones_bf = const.tile([128, 128], BF16, tag="ones_bf")
    nc.gpsimd.memset(ones_bf[:], 1.0 / O)
    zeros_bf = const.tile([128, 128], BF16, tag="zeros_bf")
    nc.gpsimd.memset(zeros_bf[:], 0.0)

    # act table preload (overlaps the x DMA)
    actpre = const.tile([128, 1], F32, tag="actpre")
    nc.gpsimd.memset(actpre[:], 0.0)
    nc.scalar.activation(actpre[:], actpre[:], AF.Exp)

    # ---- PE warmup: real matmuls so HAM un-throttles before the iter loop ----
    if WARMUP_MMS:
        warm = psumT.tile([128, B * DI], F32, tag="tps")
        for _ in range(WARMUP_MMS):
            nc.tensor.matmul(warm[:], lhsT=id_sb[:], rhs=id_sb[:, :B * DI],
                             start=True, stop=True, skip_group_check=True)

    # ---- squash ----
    xsq = big.tile([128, B * J * DI], BF16, tag="xsq")
    nc.scalar.square(xsq[:], xr[:])
    n2x = small.tile([128, B * J], F32, tag="n2x")
    nc.vector.reduce_sum(n2x[:], xsq[:].rearrange("p (r i) -> p r i", i=DI),
                         axis=mybir.AxisListType.X)
    gx = small.tile([128, B * J], F32, tag="gx")
    squash_scale(gx[:], n2x[:], "sq")
    xs_bf = big.tile([128, B * J * DI], BF16, tag="xs_bf")
    nc.vector.tensor_mul(
        xs_bf[:].rearrange("p (r i) -> p r i", i=DI),
        xr[:].rearrange("p (r i) -> p r i", i=DI),
        gx[:].unsqueeze(2).broadcast_to([128, B * J, DI]),
    )

    # ---- xsp: padded/permuted copy; 16 PE transposes -> xsT strips at rows b*32
    if DEBUG_DUMP == "xs":
        dump_stop(xs_bf[:, 0:64], "xs_bf cols 0:64")
        return

    xsp = big.tile([128, J * 128], BF16, tag="xsp")
    nc.gpsimd.memset(xsp[:], 0.0)
    nc.vector.tensor_copy(
        xsp[:].rearrange("p (j b w) -> p j b w", j=J, b=B)[:, :, :, :DI],
        xs_bf[:].rearrange("p (b j i) -> p j b i", b=B, j=J),
    )
    xsT = big.tile([128, J * 128], BF16, tag="xsT")
    for j in range(J):
        tp = psumX.tile([128, 128], BF16, tag="tp")
        nc.tensor.transpose(tp[:], xsp[:, j * 128:(j + 1) * 128], id_sb[:])
        if j % 4 == 3:
            nc.scalar.copy(xsT[:, j * 128:(j + 1) * 128], tp[:])
        else:
            nc.vector.tensor_copy(xsT[:, j * 128:(j + 1) * 128], tp[:])

    if DEBUG_DUMP == "xsT":
        dump_stop(xsT[:, 0:64], "xsT block j=0 cols 0:64")
        return

    # ---- persistent state ----
    # e layout [p, (j, b, o)]: b-pair contiguous for the paired t-matmul lhsT
    e_bf = big.tile([128, J * B * O], BF16, tag="e_bf")
    xz_bf = big.tile([128, B * J * DI], BF16, tag="xz_bf")
    wv_pad = const.tile([128, 2 * 32], BF16, tag="wv_pad")  # (q, ii32), pads zero
    nc.gpsimd.memset(wv_pad[:], 0.0)
    wv0f = const.tile([128, 2 * DI], F32, tag="wv0f")       # (q, i)
    trc = [None, None]

    for it in range(3):
        if it > 0:
            # ---- logits: strips j, batch b on quad rows b*32; batch pairs to
            # keep L at 2 PSUM banks per batch (bufs=2 -> pair in flight)
            for pr in range(2):
                Lp = [psumL.tile([128, J * O], F32, tag="L", name=f"L_{pr}_{bl}")
                      for bl in range(2)]
                for j in range(J):
                    for bl in range(2):
                        b = 2 * pr + bl
                        nc.tensor.matmul(
                            Lp[bl][:, j * O:(j + 1) * O],
                            lhsT=xsT[b * 32:b * 32 + DI, j * 128:(j + 1) * 128],
                            rhs=trc[pr][b * 32:b * 32 + DI, bl * O:(bl + 1) * O],
                            tile_position=(b * 32, 0),
                            start=True,
                            stop=True,
                        )
                if DEBUG_DUMP == f"L{it}" and pr == 0:
                    dump_stop(Lp[0][:, 0:64], f"L b0 j=0 it={it}")
                    return
                for bl in range(2):
                    b = 2 * pr + bl
                    ev = e_bf[:].rearrange("p (j b2 o) -> p b2 j o", j=J, b2=B)[:, b]
                    nc.scalar.activation(ev, Lp[bl][:], AF.Exp)
                    zb = small.tile([128, J], F32, tag="zb")
                    nc.vector.reduce_sum(zb[:], ev, axis=mybir.AxisListType.X)
                    rz = small.tile([128, J], F32, tag="rz")
                    nc.vector.reciprocal(rz[:], zb[:])
                    xzb = xz_bf[:, b * J * DI:(b + 1) * J * DI].rearrange("p (j i) -> p j i", i=DI)
                    if MIXED_TT:
                        nc.vector.tensor_mul(
                            xzb,
                            xs_bf[:, b * J * DI:(b + 1) * J * DI].rearrange("p (j i) -> p j i", i=DI),
                            rz[:].unsqueeze(2).broadcast_to([128, J, DI]),
                        )
                    else:
                        gz = small.tile([128, J], F32, tag="gz")
                        nc.vector.tensor_mul(gz[:], gx[:, b * J:(b + 1) * J], rz[:])
                        nc.vector.tensor_mul(
                            xzb,
                            xr[:, b * J * DI:(b + 1) * J * DI].rearrange("p (j i) -> p j i", i=DI),
                            gz[:].unsqueeze(2).broadcast_to([128, J, DI]),
                        )

        # ---- t matmul -> tps [128, (b, i)]; valid: b even rows 0:64, odd 64:128
        # The 4 batch regions share one PSUM bank and all 128 partitions, so a
        # start=True per region would wipe the others' has_written bits. One
        # bank-wide zero-weight clear MM instead; all real MMs accumulate.
        tps = psumT.tile([128, B * DI], F32, tag="tps")
        nc.tensor.matmul(tps[:], lhsT=zeros_bf[:], rhs=ones_bf[:, :B * DI],
                         start=True, stop=False, skip_group_check=True)
        for q in range(2):
            for j in range(J):
                lhsT = (ones_bf[:, :] if it == 0
                        else e_bf[:, (j * B + 2 * q) * O:(j * B + 2 * q + 2) * O])
                src = xs_bf if it == 0 else xz_bf
                for bl in range(2):
                    b = 2 * q + bl
                    nc.tensor.matmul(
                        tps[:, b * DI:(b + 1) * DI],
                        lhsT=lhsT,
                        rhs=src[:, (b * J + j) * DI:(b * J + j + 1) * DI],
                        start=False,
                        stop=(q == 1 and j == J - 1),
                        skip_group_check=True,
                    )

        if DEBUG_DUMP == f"t{it}":
            dump_stop(tps[:], f"tps it={it}")
            return

        # ---- t_sb [128, (q, i)]: rows 0:64 = b even, rows 64:128 = b odd ----
        t_sb = small.tile([128, 2 * DI], BF16, tag="t_sb")
        nc.scalar.copy(
            t_sb[0:64].rearrange("p (z i) -> p z i", z=2),
            tps[0:64].rearrange("p (b i) -> p b i", b=B)[:, 0::2],
        )
        nc.vector.tensor_copy(
            t_sb[64:128].rearrange("p (z i) -> p z i", z=2),
            tps[64:128].rearrange("p (b i) -> p b i", b=B)[:, 1::2],
        )

        if it < 2:
            # ---- small stage in [128, (q, i)] ----
            qm = small.tile([128, 2 * DI * DI], BF16, tag="qm")
            nc.vector.tensor_mul(
                qm[:].rearrange("p (z i j) -> p z i j", z=2, i=DI),
                m2_sb[:].rearrange("p (i j) -> p i j", j=DI).unsqueeze(1).broadcast_to([128, 2, DI, DI]),
                t_sb[:].rearrange("p (z j) -> p z j", z=2).unsqueeze(2).broadcast_to([128, 2, DI, DI]),
            )
            q_t = small.tile([128, 2 * DI], F32, tag="q_t")
            nc.vector.reduce_sum(q_t[:], qm[:].rearrange("p (r j) -> p r j", j=DI),
                                 axis=mybir.AxisListType.X)
            scr = small.tile([128, 2 * DI], F32, tag="scr")
            if MIXED_TT:
                nc.vector.tensor_mul(scr[:], q_t[:], t_sb[:])
            else:
                tf = small.tile([128, 2 * DI], F32, tag="tf")
                nc.vector.tensor_copy(tf[:], t_sb[:])
                nc.vector.tensor_mul(scr[:], q_t[:], tf[:])
            n2t = small.tile([128, 2], F32, tag="n2t")
            nc.vector.reduce_sum(n2t[:], scr[:].rearrange("p (z i) -> p z i", z=2),
                                 axis=mybir.AxisListType.X)
            h = small.tile([128, 2], F32, tag="h")
            squash_scale(h[:], n2t[:], "h")
            # wv into padded bf16 (cols ii<16 of each 32-block) + f32 running sum
            wvv = wv_pad[:].rearrange("p (z w) -> p z w", z=2)[:, :, :DI]
            if it == 0:
                nc.vector.tensor_mul(
                    wv0f[:].rearrange("p (z i) -> p z i", z=2),
                    q_t[:].rearrange("p (z i) -> p z i", z=2),
                    h[:].unsqueeze(2).broadcast_to([128, 2, DI]),
                )
                nc.vector.tensor_copy(wvv, wv0f[:].rearrange("p (z i) -> p z i", z=2))
            else:
                hq = small.tile([128, 2 * DI], F32, tag="hq")
                nc.vector.tensor_mul(
                    hq[:].rearrange("p (z i) -> p z i", z=2),
                    q_t[:].rearrange("p (z i) -> p z i", z=2),
                    h[:].unsqueeze(2).broadcast_to([128, 2, DI]),
                )
                nc.vector.tensor_add(wvv, hq[:].rearrange("p (z i) -> p z i", z=2),
                                     wv0f[:].rearrange("p (z i) -> p z i", z=2))
            # trc_q = transpose(wv_pad q-slice): 4 row-replicas each
            trcp = psumW.tile([128, 256], BF16, tag="trcp")
            for q in range(2):
                for r4 in range(4):
                    nc.tensor.transpose(
                        trcp[r4 * 32:(r4 + 1) * 32, q * 128:(q + 1) * 128],
                        wv_pad[:, q * 32:(q + 1) * 32],
                        id_sb[:],
                        tile_position=(0, r4 * 32),
                    )
            if DEBUG_DUMP == f"wv{it}":
                dump_stop(wv_pad[:], f"wv_pad it={it}")
                return
            trc0 = small.tile([128, 128], BF16, tag="trc0")
            trc1 = small.tile([128, 128], BF16, tag="trc1")
            nc.scalar.copy(trc0[:], trcp[:, 0:128])
            nc.vector.tensor_copy(trc1[:], trcp[:, 128:256])
            trc = [trc0, trc1]
            if DEBUG_DUMP == f"trc{it}":
                dump_stop(trc0[:, 0:64], f"trc0 cols 0:64 it={it}")
                return
        else:
            # ---- final: v = h * (W @ t) in [128, (q, d)] ----
            sm = small.tile([128, 2 * D * DI], BF16, tag="sm")
            nc.vector.tensor_mul(
                sm[:].rearrange("p (z d i) -> p z d i", z=2, d=D),
                w_sb[:].rearrange("p (d i) -> p d i", i=DI).unsqueeze(1).broadcast_to([128, 2, D, DI]),
                t_sb[:].rearrange("p (z i) -> p z i", z=2).unsqueeze(2).broadcast_to([128, 2, D, DI]),
            )
            s_sb = small.tile([128, 2 * D], F32, tag="s_sb")
            nc.vector.reduce_sum(s_sb[:], sm[:].rearrange("p (r i) -> p r i", i=DI),
                                 axis=mybir.AxisListType.X)
            s2 = small.tile([128, 2 * D], F32, tag="s2")
            nc.vector.tensor_mul(s2[:], s_sb[:], s_sb[:])
            n2v = small.tile([128, 2], F32, tag="n2v")
            nc.vector.reduce_sum(n2v[:], s2[:].rearrange("p (z d) -> p z d", z=2),
                                 axis=mybir.AxisListType.X)
            hv = small.tile([128, 2], F32, tag="hv")
            squash_scale(hv[:], n2v[:], "hv")
            v_sb = small.tile([128, 2 * D], F32, tag="v_sb")
            nc.vector.tensor_mul(
                v_sb[:].rearrange("p (z d) -> p z d", z=2),
                s_sb[:].rearrange("p (z d) -> p z d", z=2),
                hv[:].unsqueeze(2).broadcast_to([128, 2, D]),
            )
            # b0=v[0:64,0:32] b1=v[64:128,0:32] b2=v[0:64,32:64] b3=v[64:128,32:64]
            for b in range(B):
                nc.sync.dma_start(
                    vout[b],
                    v_sb[(b % 2) * 64:(b % 2) * 64 + 64, (b // 2) * D:(b // 2 + 1) * D],
                )
    ctx.close()


_CACHE = {}


def _get_module():
    if "nc" not in _CACHE:
        nc = bacc.Bacc("TRN2", target_bir_lowering=False, debug=False,
                       enable_asserts=False, num_devices=N_CORES)
        with tile.TileContext(nc) as tc:
            build_kernel(nc, tc)
        nc.compile()
        _CACHE["nc"] = nc
    return _CACHE["nc"]


def _host_inputs(input_vectors, weight_matrix):
    W0 = np.asarray(weight_matrix, dtype=np.float32)[0]          # [O, D, DI]
    M2 = np.einsum("odi,odj->oij", W0, W0).astype(np.float32)    # [O, DI, DI]
    wrep = np.tile(W0.reshape(O, D * DI), (2, 1)).astype(np.float16)
    m2rep = np.tile(M2.reshape(O, DI * DI), (2, 1)).astype(np.float16)
    ident = np.eye(128, dtype=np.float16)
    x = np.ascontiguousarray(np.asarray(input_vectors, dtype=np.float32))
    in_maps = []
    for c in range(N_CORES):
        in_maps.append({
            "x": np.ascontiguousarray(x[c * B:(c + 1) * B]),
            "wrep": wrep,
            "m2rep": m2rep,
            "ident": ident,
        })
    return in_maps


def run(input_vectors, weight_matrix, trace=False, tmpdir=None):
    nc = _get_module()
    in_maps = _host_inputs(input_vectors, weight_matrix)
    res = run_bass_kernel_spmd(
        nc, in_maps, core_ids=list(range(N_CORES)), trace=trace, tmpdir=tmpdir
    )
    out = np.concatenate([res.results[c]["vout"] for c in range(N_CORES)], axis=0)
    return out.astype(np.float32), res


def kernel(input_vectors, weight_matrix):
    out, _ = run(input_vectors, weight_matrix, trace=False)
    return out

